# revision 12
# baseline (speedup 1.0000x reference)
"""Trainium2 Bass kernel for nn_DenoiserBlock (B=2, L=2048, D=1024, H=16, F=4096).

Sharding: 8 cores = 2 (batch) x 4 (query-slice of 512). Each core computes
K/V for the full sequence of its batch element (data redundancy instead of
collectives), attention + MLP for its 512-query slice. Host does the
(cheap) AdaLN modulation precompute, weight re-layout/casting, and final
concatenation of the 8 [512, 1024] output slices.

Device dataflow (per core, "T" = feature-major / transposed layout):
  A: LN1+AdaLN over x[b] -> h, PE-transpose -> hT (bf16); same for the
     512 residual rows -> hresT
  B: kT = Wk^T hT, v = hT^T Wv (with an appended ones-column per head for
     softmax denominators), qT = Wq_s^T hresT
  C: per head: sT = kT_h^T qT_h (K=64 matmul), DVE fuses the torus bias,
     ACT exp, attn@v accumulated over 16 key tiles; row 64 of the psum is
     the softmax denominator; normalize via reciprocal + PE broadcast
  D: out2 = OUT^T^T Wout + x_res, LN2, transpose -> h2T
  E: aT[f-tile] = gelu(W1^T h2T + b1) for 32 f-tiles, then
     y = aT^T W2 + (x2 + b2)
"""

import sys

sys.path.insert(0, "/opt/trn_rl_repo")

import numpy as np
import ml_dtypes

import concourse.bacc as bacc
import concourse.mybir as mybir
from concourse import tile, masks
from concourse.bass_utils import run_bass_kernel_spmd

F32 = mybir.dt.float32
BF16 = mybir.dt.bfloat16
F32R = mybir.dt.float32r
AX = mybir.AxisListType
OP = mybir.AluOpType
ACT = mybir.ActivationFunctionType

B, L, D, H, F = 2, 2048, 1024, 16, 4096
HD = D // H          # 64
QS = 512             # queries per core
NC_PER_B = 4
EPS = 1e-5

_CACHED = {}


def _build(shared_mask=True):
    nc = bacc.Bacc("TRN2", target_bir_lowering=False, debug=False, num_devices=8)

    d_x = nc.dram_tensor("x_full", [L, D], F32, kind="ExternalInput")
    d_xres = nc.dram_tensor("x_res", [QS, D], F32, kind="ExternalInput")
    if shared_mask:
        d_expm = nc.dram_tensor("expm", [L, QS], BF16, kind="ExternalInput")
    else:
        d_expm = nc.dram_tensor("expm", [H, L, QS], BF16, kind="ExternalInput")
    d_wq = nc.dram_tensor("wq", [D, D], BF16, kind="ExternalInput")
    d_wk = nc.dram_tensor("wk", [D, D], BF16, kind="ExternalInput")
    d_wv = nc.dram_tensor("wv", [D, D], BF16, kind="ExternalInput")
    d_wout = nc.dram_tensor("wout", [D, D], BF16, kind="ExternalInput")
    d_w1t = nc.dram_tensor("w1t", [32, 8, 128, 128], BF16, kind="ExternalInput")
    d_w2 = nc.dram_tensor("w2", [F, D], BF16, kind="ExternalInput")
    d_bias2r = nc.dram_tensor("bias2r", [128, D], F32, kind="ExternalInput")
    d_biask = nc.dram_tensor("biask", [128, 8], F32, kind="ExternalInput")
    d_biasq = nc.dram_tensor("biasq", [128, 8], F32, kind="ExternalInput")
    d_bvrep = nc.dram_tensor("bvrep", [128, D], F32, kind="ExternalInput")
    d_b1sb = nc.dram_tensor("b1sb", [128, 32], F32, kind="ExternalInput")
    d_y = nc.dram_tensor("y", [QS, D], F32, kind="ExternalOutput")

    NLT = L // 128
    NDT = D // 128
    NQT = QS // 128
    NFT = F // 128

    with tile.TileContext(nc) as tc:
        with (
            tc.tile_pool(name="const", bufs=1) as cpool,
            tc.tile_pool(name="mid", bufs=1) as mpool,
            tc.tile_pool(name="psum", bufs=1, space="PSUM") as pspool,
        ):
            b1sb = cpool.tile([128, 32], F32, tag="b1sb")
            ident = cpool.tile([128, 128], BF16, tag="ident")
            epsc = cpool.tile([128, 1], F32, tag="epsc")
            biask = cpool.tile([128, 8], F32, tag="biask")
            biasq = cpool.tile([128, 8], F32, tag="biasq")
            bvrep = cpool.tile([128, D], F32, tag="bvrep")
            nc.sync.dma_start(b1sb[:], d_b1sb[:, :])
            nc.sync.dma_start(biask[:], d_biask[:, :])
            nc.sync.dma_start(biasq[:], d_biasq[:, :])
            nc.sync.dma_start(bvrep[:], d_bvrep[:, :])
            masks.make_identity(nc, ident[:])
            nc.vector.memset(epsc[:], EPS)

            outT = [mpool.tile([128, QS], BF16, tag=f"outT{i}", name=f"outT{i}")
                    for i in range(NDT)]
            x2 = [mpool.tile([128, D], F32, tag=f"x2{i}", name=f"x2{i}")
                  for i in range(NQT)]
            h2T = [mpool.tile([128, QS], BF16, tag=f"h2T{i}", name=f"h2T{i}")
                   for i in range(NDT)]

            def layer_norm_tile(pool, pspool, xt, hT_tiles, col0):
                """Normalize one [128, D] tile (no gain/bias - folded into the
                consuming weights host-side) -> bf16 transposed blocks into
                hT_tiles[j][:, col0:col0+128]."""
                s1 = pool.tile([128, 1], F32, tag="lns", name="s1")
                s2 = pool.tile([128, 1], F32, tag="lns", name="s2")
                mu = pool.tile([128, 1], F32, tag="lns", name="mu")
                msq = pool.tile([128, 1], F32, tag="lns", name="msq")
                var = pool.tile([128, 1], F32, tag="lns", name="var")
                std = pool.tile([128, 1], F32, tag="lns", name="std")
                rstd = pool.tile([128, 1], F32, tag="lns", name="rstd")
                sq = pool.tile([128, D], F32, tag="xc", name="sq")
                hb = pool.tile([128, D], BF16, tag="hb", name="hb")
                nc.vector.tensor_reduce(s1[:], xt[:], axis=AX.X, op=OP.add)
                nc.scalar.activation(sq[:], xt[:], ACT.Square, accum_out=s2[:])
                nc.scalar.mul(mu[:], s1[:], 1.0 / D)
                nc.vector.tensor_tensor(msq[:], mu[:], mu[:], op=OP.mult)
                nc.vector.scalar_tensor_tensor(
                    var[:], s2[:], 1.0 / D, msq[:], op0=OP.mult, op1=OP.subtract)
                nc.scalar.activation(std[:], var[:], ACT.Sqrt, bias=epsc[:])
                nc.vector.reciprocal(rstd[:], std[:])
                nc.vector.tensor_scalar(hb[:], xt[:], mu[:], rstd[:],
                                        op0=OP.subtract, op1=OP.mult)
                for j in range(NDT):
                    pt = pspool.tile([128, 128], BF16, tag="trp", name="trp", bufs=2)
                    nc.tensor.transpose(pt[:], hb[:, j * 128:(j + 1) * 128], ident[:])
                    if j % 2 == 0:
                        nc.scalar.copy(hT_tiles[j][:, col0:col0 + 128], pt[:])
                    else:
                        nc.vector.tensor_copy(hT_tiles[j][:, col0:col0 + 128], pt[:])

            with tc.tile_pool(name="attn", bufs=1) as atpool:
                kT = [atpool.tile([128, L], BF16, tag=f"kT{i}", name=f"kT{i}")
                      for i in range(NDT)]
                vv = [atpool.tile([128, H * (HD + 1)], BF16, tag=f"v{i}", name=f"v{i}")
                      for i in range(NLT)]
                qT = [atpool.tile([128, QS], BF16, tag=f"qT{i}", name=f"qT{i}")
                      for i in range(NDT)]

                # ---- Phase A ----
                with tc.tile_pool(name="hTp", bufs=1) as hpool:
                    hT = [hpool.tile([128, L], BF16, tag=f"hT{i}", name=f"hT{i}")
                          for i in range(NDT)]
                    hresT = [hpool.tile([128, QS], BF16, tag=f"hrT{i}", name=f"hrT{i}")
                             for i in range(NDT)]
                    with tc.tile_pool(name="phA", bufs=5) as apool:
                        for lt in range(NLT):
                            xt = apool.tile([128, D], F32, tag="xt", name="xt", bufs=2)
                            nc.sync.dma_start(xt[:], d_x[lt * 128:(lt + 1) * 128, :])
                            layer_norm_tile(apool, pspool, xt, hT, lt * 128)
                        for rt in range(NQT):
                            xt = apool.tile([128, D], F32, tag="xt", name="xt", bufs=2)
                            nc.sync.dma_start(xt[:], d_xres[rt * 128:(rt + 1) * 128, :])
                            layer_norm_tile(apool, pspool, xt, hresT, rt * 128)

                    # ---- Phase B ----
                    with tc.tile_pool(name="wtsQ", bufs=1) as wqpool:
                        wq = [wqpool.tile([128, D], BF16, tag=f"wq{i}", name=f"wq{i}")
                              for i in range(NDT)]
                        for i in range(NDT):
                            nc.sync.dma_start(wq[i][:], d_wq[i * 128:(i + 1) * 128, :])
                        for i in range(NDT):
                            pq = pspool.tile([128, 512], F32, tag="mm", name="pq", bufs=4)
                            for dt_ in range(NDT):
                                nc.tensor.matmul(
                                    pq[:], wq[dt_][:, i * 128:(i + 1) * 128],
                                    hresT[dt_][:],
                                    start=(dt_ == 0), stop=(dt_ == NDT - 1))
                            nc.vector.tensor_scalar(qT[i][:], pq[:], biasq[:, i:i + 1],
                                                    None, op0=OP.add)

                    with tc.tile_pool(name="wtsK", bufs=1) as wkpool:
                        wk = [wkpool.tile([128, D], BF16, tag=f"wk{i}", name=f"wk{i}")
                              for i in range(NDT)]
                        for i in range(NDT):
                            nc.sync.dma_start(wk[i][:], d_wk[i * 128:(i + 1) * 128, :])
                        for i in range(NDT):
                            for ncol in range(L // 512):
                                pk = pspool.tile([128, 512], F32, tag="mm", name="pk", bufs=4)
                                for dt_ in range(NDT):
                                    nc.tensor.matmul(
                                        pk[:], wk[dt_][:, i * 128:(i + 1) * 128],
                                        hT[dt_][:, ncol * 512:(ncol + 1) * 512],
                                        start=(dt_ == 0), stop=(dt_ == NDT - 1))
                                nc.vector.tensor_scalar(
                                    kT[i][:, ncol * 512:(ncol + 1) * 512], pk[:],
                                    biask[:, i:i + 1], None, op0=OP.add)

                    with tc.tile_pool(name="wtsV", bufs=1) as wvpool:
                        wv = [wvpool.tile([128, D], BF16, tag=f"wv{i}", name=f"wv{i}")
                              for i in range(NDT)]
                        for i in range(NDT):
                            nc.sync.dma_start(wv[i][:], d_wv[i * 128:(i + 1) * 128, :])
                        for lt in range(NLT):
                            v3 = vv[lt][:].rearrange("p (h c) -> p h c", c=HD + 1)
                            for half in range(2):
                                pv = pspool.tile([128, 512], F32, tag="mm", name="pv", bufs=4)
                                for dt_ in range(NDT):
                                    nc.tensor.matmul(
                                        pv[:], hT[dt_][:, lt * 128:(lt + 1) * 128],
                                        wv[dt_][:, half * 512:(half + 1) * 512],
                                        start=(dt_ == 0), stop=(dt_ == NDT - 1))
                                nc.vector.tensor_tensor(
                                    v3[:, half * 8:(half + 1) * 8, 0:HD], pv[:],
                                    bvrep[:, half * 512:(half + 1) * 512], op=OP.add)
                            nc.vector.memset(v3[:, :, HD:HD + 1], 1.0)

                # ---- Phase C ----
                with (
                    tc.tile_pool(name="phC", bufs=6) as cwork,
                    tc.tile_pool(name="mres", bufs=1) as mpool_c,
                ):
                    mres = None
                    if shared_mask:
                        mres = [mpool_c.tile([128, QS], BF16, tag=f"mr{i}",
                                             name=f"mr{i}") for i in range(NLT)]
                        for kt in range(NLT):
                            nc.sync.dma_start(
                                mres[kt][:], d_expm[kt * 128:(kt + 1) * 128, :])
                    for hp in range(H // 2):
                        ht = hp
                        pos = [pspool.tile([65, 512], F32, tag="acc",
                                           name=f"po{par}", bufs=2) for par in range(2)]
                        for kt in range(NLT):
                            for par in range(2):
                                h, ho = 2 * hp + par, par * 64
                                if shared_mask:
                                    mt = mres[kt]
                                else:
                                    mt = cwork.tile([128, 512], BF16, tag="mt",
                                                    name="mt")
                                    nc.sync.dma_start(
                                        mt[:], d_expm[h, kt * 128:(kt + 1) * 128, :])
                                ps = pspool.tile([128, 512], F32, tag="mm",
                                                 name="ps", bufs=4)
                                nc.tensor.matmul(
                                    ps[:], kT[ht][ho:ho + 64, kt * 128:(kt + 1) * 128],
                                    qT[ht][ho:ho + 64, :], start=True, stop=True)
                                pb = cwork.tile([128, 512], BF16, tag="pb", name="pb")
                                nc.scalar.activation(pb[:], ps[:], ACT.Exp)
                                pm = cwork.tile([128, 512], BF16, tag="pm", name="pm")
                                nc.vector.tensor_tensor(pm[:], pb[:], mt[:],
                                                        op=OP.mult)
                                v3 = vv[kt][:].rearrange("p (h c) -> p h c", c=HD + 1)
                                nc.tensor.matmul(
                                    pos[par][:], v3[:, h, :], pm[:],
                                    start=(kt == 0), stop=(kt == NLT - 1))
                        for par in range(2):
                            ho = par * 64
                            recip = cwork.tile([1, 512], F32, tag="recip",
                                               name="recip")
                            nc.vector.reciprocal(recip[:], pos[par][64:65, :])
                            rbs = cwork.tile([64, 512], F32, tag="rbs", name="rbs")
                            nc.gpsimd.partition_broadcast(rbs[:], recip[:])
                            nc.vector.tensor_tensor(
                                outT[ht][ho:ho + 64, :], pos[par][0:64, :], rbs[:],
                                op=OP.mult)

                # ---- Phase D ----
                with (
                    tc.tile_pool(name="phD", bufs=4) as dwork,
                    tc.tile_pool(name="phD_w", bufs=1) as dwpool,
                ):
                    bias2r = dwpool.tile([128, D], F32, tag="bias2r")
                    nc.sync.dma_start(bias2r[:], d_bias2r[:, :])
                    wo = [dwpool.tile([128, D], BF16, tag=f"wo{i}", name=f"wo{i}")
                          for i in range(NDT)]
                    for i in range(NDT):
                        nc.sync.dma_start(wo[i][:], d_wout[i * 128:(i + 1) * 128, :])
                    xr = [dwpool.tile([128, D], F32, tag=f"xr{i}", name=f"xr{i}")
                          for i in range(NQT)]
                    for i in range(NQT):
                        nc.sync.dma_start(xr[i][:], d_xres[i * 128:(i + 1) * 128, :])
                    for qt in range(NQT):
                        for half in range(2):
                            p2 = pspool.tile([128, 512], F32, tag="mm", name="p2", bufs=4)
                            for dt_ in range(NDT):
                                nc.tensor.matmul(
                                    p2[:], outT[dt_][:, qt * 128:(qt + 1) * 128],
                                    wo[dt_][:, half * 512:(half + 1) * 512],
                                    start=(dt_ == 0), stop=(dt_ == NDT - 1))
                            nc.vector.tensor_tensor(
                                x2[qt][:, half * 512:(half + 1) * 512], p2[:],
                                xr[qt][:, half * 512:(half + 1) * 512], op=OP.add)
                        layer_norm_tile(dwork, pspool, x2[qt], h2T, qt * 128)
                        nc.vector.tensor_tensor(x2[qt][:], x2[qt][:], bias2r[:],
                                                op=OP.add)

            # ---- Phase E ----
            with (
                tc.tile_pool(name="phE_a", bufs=1) as e_apool,
                tc.tile_pool(name="phE_w", bufs=4) as e_wpool,
                tc.tile_pool(name="phE_w2", bufs=1) as e_w2pool,
                tc.tile_pool(name="phE", bufs=3) as e_work,
            ):
                aT = [e_apool.tile([128, QS], BF16, tag=f"aT{i}", name=f"aT{i}")
                      for i in range(NFT)]
                w2sb = [e_w2pool.tile([128, D], BF16, tag=f"w2_{i}", name=f"w2_{i}")
                        for i in range(NFT)]
                for ft in range(NFT):
                    nc.sync.dma_start(w2sb[ft][:], d_w2[ft * 128:(ft + 1) * 128, :])
                for ft in range(NFT):
                    w1b = e_wpool.tile([128, D], BF16, tag="w1b", name="w1b")
                    nc.sync.dma_start(
                        w1b[:].rearrange("p (d c) -> p d c", c=128),
                        d_w1t[ft].rearrange("d r c -> r d c"))
                    pa = pspool.tile([128, 512], F32, tag="mm", name="pa", bufs=4)
                    for dt_ in range(NDT):
                        nc.tensor.matmul(
                            pa[:], w1b[:, dt_ * 128:(dt_ + 1) * 128], h2T[dt_][:],
                            start=(dt_ == 0), stop=(dt_ == NDT - 1))
                    nc.scalar.activation(aT[ft][:], pa[:], ACT.Gelu_apprx_tanh,
                                         bias=b1sb[:, ft:ft + 1])
                for qt in range(NQT):
                    ysb = e_work.tile([128, D], F32, tag="ysb", name="ysb")
                    for half in range(2):
                        p3 = pspool.tile([128, 512], F32, tag="acc", name="p3", bufs=2)
                        for ft in range(NFT):
                            nc.tensor.matmul(
                                p3[:], aT[ft][:, qt * 128:(qt + 1) * 128],
                                w2sb[ft][:, half * 512:(half + 1) * 512],
                                start=(ft == 0), stop=(ft == NFT - 1))
                        nc.vector.tensor_tensor(
                            ysb[:, half * 512:(half + 1) * 512], p3[:],
                            x2[qt][:, half * 512:(half + 1) * 512], op=OP.add)
                    nc.sync.dma_start(d_y[qt * 128:(qt + 1) * 128, :], ysb[:])

    nc.compile()
    return nc


def _gelu_tanh(x):
    x = x.astype(np.float64)
    return 0.5 * x * (1.0 + np.tanh(np.sqrt(2.0 / np.pi) * (x + 0.044715 * x ** 3)))


def kernel(x, torus_dist, time_emb, mask, ln1_g, ln1_b, Wqkv, Wout,
           torus_scale, ln2_g, ln2_b, W1, b1, W2, b2, Wt, bt):
    x = np.asarray(x, np.float32)
    torus_dist = np.asarray(torus_dist, np.float32)
    time_emb = np.asarray(time_emb, np.float32)
    mask = np.asarray(mask)
    Wqkv = np.asarray(Wqkv, np.float32)

    sc_arr = np.asarray(torus_scale, np.float32)
    shared = bool(np.all(sc_arr == sc_arr[0]))
    key = f"nc_{shared}"
    if key not in _CACHED:
        _CACHED[key] = _build(shared_mask=shared)
    nc = _CACHED[key]

    bf = lambda a: np.ascontiguousarray(a).astype(ml_dtypes.bfloat16)
    rep = lambda v: np.ascontiguousarray(
        np.tile(np.asarray(v, np.float32)[None, :], (128, 1)))

    tp = (_gelu_tanh(time_emb) @ np.asarray(Wt, np.float64)
          + np.asarray(bt, np.float64))          # [B, 2D]
    scale, shift = tp[:, :D], tp[:, D:]
    g_eff = (np.asarray(ln1_g, np.float64)[None, :] * (1.0 + scale)).astype(np.float32)
    b_eff = (np.asarray(ln1_b, np.float64)[None, :] * (1.0 + scale) + shift).astype(np.float32)

    Wq_r = np.asarray(Wqkv[:, 0:D], np.float64) / np.sqrt(64.0)
    Wk_r = np.asarray(Wqkv[:, D:2 * D], np.float64)
    Wv_r = np.asarray(Wqkv[:, 2 * D:3 * D], np.float64)
    W1_r = np.asarray(W1, np.float64)
    g2 = np.asarray(ln2_g, np.float64)
    b2ln = np.asarray(ln2_b, np.float64)
    w1t_g = (g2[:, None] * W1_r).astype(np.float32)
    w1t = bf(w1t_g.reshape(8, 128, 32, 128).transpose(2, 0, 1, 3))
    b1sb_eff = (np.asarray(b1, np.float64) + b2ln @ W1_r).astype(np.float32)
    b1sb = np.ascontiguousarray(b1sb_eff.reshape(32, 128).T)
    w2 = bf(W2)
    wout = bf(Wout)
    bias2r = rep(b2)

    in_maps = []
    for c in range(8):
        b_, qs_ = c // NC_PER_B, c % NC_PER_B
        rows = slice(qs_ * QS, (qs_ + 1) * QS)
        km = np.where(mask[b_], 0.0, -88.0).astype(np.float32)      # [L]
        torT = torus_dist[0, rows, :].T.astype(np.float32)           # [L, QS]
        if shared:
            expm = np.exp(km[:, None] - sc_arr[0] * torT).astype(ml_dtypes.bfloat16)
        else:
            expm = np.exp(km[None, :, None] - sc_arr[:, None, None]
                          * torT[None, :, :]).astype(ml_dtypes.bfloat16)
        ge = g_eff[b_].astype(np.float64)
        be = b_eff[b_].astype(np.float64)
        wq_b = bf((ge[:, None] * Wq_r).astype(np.float32))
        wk_b = bf((ge[:, None] * Wk_r).astype(np.float32))
        wv_b = bf((ge[:, None] * Wv_r).astype(np.float32))
        bk = (be @ Wk_r).astype(np.float32)
        bq = (be @ Wq_r).astype(np.float32)
        bv = (be @ Wv_r).astype(np.float32)
        in_maps.append({
            "x_full": x[b_],
            "x_res": np.ascontiguousarray(x[b_, rows]),
            "expm": expm,
            "wq": wq_b, "wk": wk_b, "wv": wv_b, "wout": wout,
            "w1t": w1t, "w2": w2,
            "biask": np.ascontiguousarray(bk.reshape(8, 128).T),
            "biasq": np.ascontiguousarray(bq.reshape(8, 128).T),
            "bvrep": rep(bv),
            "bias2r": bias2r, "b1sb": b1sb,
        })

    import os
    trace = bool(int(os.environ.get("DENOISER_TRACE", "0")))
    res = run_bass_kernel_spmd(nc, in_maps, core_ids=list(range(8)), trace=trace)
    _CACHED["last_results"] = res

    out = np.empty((B, L, D), np.float32)
    for c in range(8):
        b_, qs_ = c // NC_PER_B, c % NC_PER_B
        out[b_, qs_ * QS:(qs_ + 1) * QS, :] = res.results[c]["y"]
    return out
